# revision 1
# baseline (speedup 1.0000x reference)
"""Trainium2 Bass kernel for nn_Decoder (6-layer transformer decoder, D=512, H=8,
S=128, M=196, V=32000, B=16) on 8 NeuronCores.

Sharding: data-parallel trunk over batch (2 sequences/core); the final logit
projection is vocab-sharded (V padded to 32768 -> 4096 cols/core) after an
AllGather of the final hidden states.

On-device layout: activations kept transposed ([d, token]) end to end so every
linear is lhsT=W-chunk, rhs=xT-chunk with fp32r matmuls (full PE rate at free
dim >= 256, ~1e-4 rounding). LayerNorm / softmax partition-dim reductions use
PE ones-matmuls; per-token stats are broadcast across partitions with
gpsimd.partition_broadcast. The value bias is folded into the attention output
(softmax rows sum to 1), which keeps V in natural layout bias-free.
"""

import functools
import os
from contextlib import ExitStack

import numpy as np

import concourse.bass as bass
import concourse.tile as tile
from concourse import bacc, library_config, mybir
from concourse.bass_utils import run_bass_kernel_spmd

F32 = mybir.dt.float32
F32R = mybir.dt.float32r
AF = mybir.ActivationFunctionType
ALU = mybir.AluOpType

D, H, L, V, B, S, M, MAXLEN = 512, 8, 6, 32000, 16, 128, 196, 256
DK = D // H
FF = 4 * D
N_CORES = 8
SEQ_PER_CORE = B // N_CORES          # 2
TOK = SEQ_PER_CORE * S               # 256 tokens per core
NTOK = B * S                         # 2048 total tokens
VPAD = 32768
VSH = VPAD // N_CORES                # 4096 vocab cols per core
NDC = D // 128                       # 4 d-chunks
NFC = FF // 128                      # 16 ff-chunks

# --- packed per-layer vector params (biases / ln params), host-transposed to
# [128, NCOL] so each (param, layer, chunk) is one column ---------------------
_PARAMS_D = ["sa_qb", "sa_kb", "sa_vb", "sa_ob", "sa_lng", "sa_lnb",
             "ca_qb", "ca_kb", "ca_vb", "ca_ob", "ca_lng", "ca_lnb",
             "ff_lng", "ff_lnb", "ff_b2"]
_COL = {}
_off = 0
for _p in _PARAMS_D:
    _COL[_p] = _off
    _off += L * NDC
_COL["ff_b1"] = _off
_off += L * NFC
_COL["logit_b"] = _off
_off += VSH // 128
NCOL = _off


def _col(param, l, c):
    if param == "ff_b1":
        return _COL[param] + l * NFC + c
    if param == "logit_b":
        return _COL[param] + c
    return _COL[param] + l * NDC + c


def build_module(n_cores=N_CORES):
    nc = bacc.Bacc("TRN2", target_bir_lowering=False, debug=False,
                   num_devices=n_cores)

    h0T = nc.dram_tensor("h0T", [D, TOK], F32R, kind="ExternalInput")
    memT_d = nc.dram_tensor("memT", [D, SEQ_PER_CORE * M], F32R,
                            kind="ExternalInput")
    maskm_d = nc.dram_tensor("maskm", [S, 2 * S], F32, kind="ExternalInput")
    vecs_d = nc.dram_tensor("vecs", [128, NCOL], F32, kind="ExternalInput")
    ones_d = nc.dram_tensor("ones", [128, 1], F32R, kind="ExternalInput")
    wd = {}
    for p in ("sa", "ca"):
        for nm in ("qw", "kw", "vw", "ow"):
            wd[f"{p}_{nm}"] = nc.dram_tensor(f"{p}_{nm}", [L, D, D], F32R,
                                             kind="ExternalInput")
    wd["ff_w1"] = nc.dram_tensor("ff_w1", [L, D, FF], F32R, kind="ExternalInput")
    wd["ff_w2"] = nc.dram_tensor("ff_w2", [L, FF, D], F32R, kind="ExternalInput")
    lw_d = nc.dram_tensor("logit_w", [D, VSH], F32R, kind="ExternalInput")
    out_d = nc.dram_tensor("logitsT", [VSH, NTOK], F32, kind="ExternalOutput")

    with tile.TileContext(nc) as tc:
        _emit(nc, tc, n_cores, h0T, memT_d, maskm_d, vecs_d, ones_d, wd,
              lw_d, out_d)
    nc.compile()
    return nc


def _emit(nc, tc, n_cores, h0T, memT_d, maskm_d, vecs_d, ones_d, wd, lw_d,
          out_d):
    nc.gpsimd.load_library(library_config.attnmlp)

    outer = ExitStack()
    with outer:
        const = outer.enter_context(tc.tile_pool(name="const", bufs=1))
        # PSUM pools live for the whole kernel: 4 + 2 + 2 = 8 banks.
        pA = outer.enter_context(tc.tile_pool(name="pA", bufs=4, space="PSUM"))
        pB = outer.enter_context(tc.tile_pool(name="pB", bufs=2, space="PSUM"))
        pC = outer.enter_context(tc.tile_pool(name="pC", bufs=2, space="PSUM"))

        vecs = const.tile([128, NCOL], F32)
        nc.sync.dma_start(vecs[:], vecs_d[:])
        ones = const.tile([128, 1], F32R)
        nc.sync.dma_start(ones[:], ones_d[:])
        maskm = const.tile([S, 2 * S], F32)
        nc.sync.dma_start(maskm[:], maskm_d[:])
        eps_t = {}
        for ev in (1e-8, 1e-6):
            et = const.tile([1, 1], F32, name=f"eps_{ev:.0e}")
            nc.vector.memset(et[:], ev)
            eps_t[ev] = et

        hT = [const.tile([128, TOK], F32R, name=f"h0T_{c}")
              for c in range(NDC)]
        for c in range(NDC):
            nc.sync.dma_start(hT[c][:], h0T[c * 128:(c + 1) * 128, :])
        memT = [const.tile([128, SEQ_PER_CORE * M], F32R,
                           name=f"memT_{c}") for c in range(NDC)]
        for c in range(NDC):
            nc.sync.dma_start(memT[c][:], memT_d[c * 128:(c + 1) * 128, :])

        es = ExitStack()
        with es:
            hpool = es.enter_context(tc.tile_pool(name="hpool", bufs=5))
            apool = es.enter_context(tc.tile_pool(name="apool", bufs=5))
            vpool = es.enter_context(tc.tile_pool(name="vpool", bufs=5))
            epool = es.enter_context(tc.tile_pool(name="epool", bufs=4))
            fpool = es.enter_context(tc.tile_pool(name="fpool", bufs=17))
            spool = es.enter_context(tc.tile_pool(name="spool", bufs=2))
            bpool = es.enter_context(tc.tile_pool(name="bpool", bufs=4))
            w512 = es.enter_context(tc.tile_pool(name="w512", bufs=9))
            w2048 = es.enter_context(tc.tile_pool(name="w2048", bufs=4))

            def load_w(dram, l, din, dout):
                pool = w2048 if dout == FF else w512
                ts = []
                for ic in range(din // 128):
                    t = pool.tile([128, dout], F32R, tag=f"w{dout}")
                    nc.sync.dma_start(t[:], dram[l, ic * 128:(ic + 1) * 128, :])
                    ts.append(t)
                return ts

            def linearT(w_tiles, xT, dout, n_free, bias_col, tag):
                nin = len(xT)
                noc = dout // 128
                pss = [pA.tile([128, n_free], F32, tag="m256",
                               name=f"lps{oc}") for oc in range(noc)]
                for ic in range(nin):
                    for oc in range(noc):
                        nc.tensor.matmul(
                            pss[oc][:], w_tiles[ic][:, oc * 128:(oc + 1) * 128],
                            xT[ic][:], start=(ic == 0), stop=(ic == nin - 1))
                outs = []
                for oc in range(noc):
                    o = apool.tile([128, n_free], F32R, tag=tag)
                    nc.any.tensor_scalar_add(
                        o[:], pss[oc][:], vecs[:, bias_col + oc:bias_col + oc + 1])
                    outs.append(o)
                return outs

            def layernormT(xT, lng_col, lnb_col, eps, tag):
                sq = []
                for c in range(NDC):
                    s = epool.tile([128, TOK], F32R, tag="lnsq")
                    nc.scalar.activation(s[:], xT[c][:], AF.Square)
                    sq.append(s)
                ssum = pC.tile([1, TOK], F32, tag="row")
                ssq = pC.tile([1, TOK], F32, tag="row")
                for c in range(NDC):
                    nc.tensor.matmul(ssum[:], ones[:, 0:1], xT[c][:],
                                     start=(c == 0), stop=(c == NDC - 1))
                for c in range(NDC):
                    nc.tensor.matmul(ssq[:], ones[:, 0:1], sq[c][:],
                                     start=(c == 0), stop=(c == NDC - 1))
                mean = spool.tile([1, TOK], F32, tag="st", bufs=8)
                nc.vector.tensor_scalar_mul(mean[:], ssum[:], 1.0 / D)
                m2 = spool.tile([1, TOK], F32, tag="st", bufs=8)
                nc.vector.tensor_tensor(m2[:], mean[:], mean[:], ALU.mult)
                var = spool.tile([1, TOK], F32, tag="st", bufs=8)
                nc.vector.scalar_tensor_tensor(var[:], ssq[:], 1.0 / D, m2[:],
                                               ALU.mult, ALU.subtract)
                sd = spool.tile([1, TOK], F32, tag="st", bufs=8)
                nc.scalar.activation(sd[:], var[:], AF.Sqrt, bias=eps_t[eps][:])
                rstd = spool.tile([1, TOK], F32, tag="st", bufs=8)
                nc.vector.reciprocal(rstd[:], sd[:])
                mr = spool.tile([1, TOK], F32, tag="st", bufs=8)
                nc.vector.tensor_tensor(mr[:], mean[:], rstd[:], ALU.mult)
                rstd_b = bpool.tile([128, TOK], F32, tag="lnb")
                nc.gpsimd.partition_broadcast(rstd_b[:], rstd[:])
                mr_b = bpool.tile([128, TOK], F32, tag="lnb")
                nc.gpsimd.partition_broadcast(mr_b[:], mr[:])
                outs = []
                for c in range(NDC):
                    t1 = epool.tile([128, TOK], F32, tag="lnt")
                    nc.vector.tensor_tensor(t1[:], xT[c][:], rstd_b[:], ALU.mult)
                    t2 = epool.tile([128, TOK], F32, tag="lnt")
                    nc.vector.tensor_tensor(t2[:], t1[:], mr_b[:], ALU.subtract)
                    o = hpool.tile([128, TOK], F32R, tag=tag)
                    nc.vector.tensor_scalar(
                        o[:], t2[:], vecs[:, lng_col + c:lng_col + c + 1],
                        vecs[:, lnb_col + c:lnb_col + c + 1], ALU.mult, ALU.add)
                    outs.append(o)
                return outs

            def attention(qT, kT, v_nat, k_sizes, k_offs, masked, vb_col):
                """qT: 4x[128,TOK]; kT: 4x[128,*] (dk x ktok); v_nat:
                per (seq, ktile) natural-layout [sz, 512] tiles.
                k_offs[j][b]: free-dim offset of k-tile j of seq b in kT.
                vb_col: value-bias column base (folded post-softmax).
                HW rule: one operand partition base per PSUM bank, so each
                head g gets its own score/PV banks."""
                outs = []
                n_kt = len(k_sizes)
                for r in range(4):
                    Es = []          # [j][g] -> E tile [sz, 2S]
                    cs = pC.tile([1, 4 * S], F32, tag="row")
                    for j in range(n_kt):
                        sz = k_sizes[j]
                        Eg = []
                        for g in range(2):
                            Sp = pA.tile([128, 2 * S], F32, tag="m256",
                                         name=f"sc{g}")
                            for b in range(SEQ_PER_CORE):
                                nc.tensor.matmul(
                                    Sp[0:sz, b * S:(b + 1) * S],
                                    kT[r][g * 64:(g + 1) * 64,
                                          k_offs[j][b]:k_offs[j][b] + sz],
                                    qT[r][g * 64:(g + 1) * 64, b * S:(b + 1) * S],
                                    start=(b == 0), stop=(b == SEQ_PER_CORE - 1),
                                    skip_group_check=True)
                            E = epool.tile([128, 2 * S], F32R, tag="E", bufs=8)
                            nc.scalar.activation(E[0:sz, :], Sp[0:sz, :], AF.Exp,
                                                 scale=1.0 / DK)
                            if masked:
                                Em = epool.tile([128, 2 * S], F32R, tag="E",
                                                bufs=8)
                                nc.vector.tensor_tensor(Em[0:sz, :], E[0:sz, :],
                                                        maskm[0:sz, :], ALU.mult)
                                E = Em
                            Eg.append(E)
                        Es.append(Eg)
                    for g in range(2):
                        for j in range(n_kt):
                            sz = k_sizes[j]
                            nc.tensor.matmul(
                                cs[0:1, g * 2 * S:(g + 1) * 2 * S],
                                ones[0:sz, 0:1], Es[j][g][0:sz, :],
                                start=(j == 0), stop=(j == n_kt - 1),
                                skip_group_check=True)
                    recip = spool.tile([1, 4 * S], F32, tag="rc", bufs=3)
                    nc.vector.reciprocal(recip[:], cs[:])
                    rb = bpool.tile([128, 4 * S], F32, tag="rb")
                    nc.gpsimd.partition_broadcast(rb[:], recip[:])
                    Pg = [pA.tile([64, TOK], F32, tag="m256", name=f"pv{g}")
                          for g in range(2)]
                    for j in range(n_kt):
                        sz = k_sizes[j]
                        for g in range(2):
                            A = epool.tile([128, 2 * S], F32R, tag="E", bufs=8)
                            nc.vector.tensor_tensor(
                                A[0:sz, :], Es[j][g][0:sz, :],
                                rb[0:sz, g * 2 * S:(g + 1) * 2 * S], ALU.mult)
                            for b in range(SEQ_PER_CORE):
                                vt = v_nat[b * n_kt + j]
                                nc.tensor.matmul(
                                    Pg[g][0:64, b * S:(b + 1) * S],
                                    vt[0:sz, (2 * r + g) * 64:(2 * r + g) * 64 + 64],
                                    A[0:sz, b * S:(b + 1) * S],
                                    start=(j == 0 and b == 0),
                                    stop=(j == n_kt - 1 and b == SEQ_PER_CORE - 1),
                                    skip_group_check=True)
                    o = apool.tile([128, TOK], F32R, tag="aT")
                    for g in range(2):
                        nc.any.tensor_scalar_add(
                            o[g * 64:(g + 1) * 64, :], Pg[g][0:64, :],
                            vecs[g * 64:(g + 1) * 64, vb_col + r:vb_col + r + 1])
                    outs.append(o)
                return outs

            def residual_ln(w_tiles, xT, bias_col, res, lng_col, lnb_col, eps,
                            tag):
                nin = len(xT)
                pss = [pA.tile([128, TOK], F32, tag="m256",
                               name=f"rps{oc}") for oc in range(NDC)]
                for ic in range(nin):
                    for oc in range(NDC):
                        nc.tensor.matmul(
                            pss[oc][:], w_tiles[ic][:, oc * 128:(oc + 1) * 128],
                            xT[ic][:], start=(ic == 0), stop=(ic == nin - 1))
                sums = []
                for oc in range(NDC):
                    sm = epool.tile([128, TOK], F32R, tag="sums")
                    nc.vector.scalar_tensor_tensor(
                        sm[:], pss[oc][:], vecs[:, bias_col + oc:bias_col + oc + 1],
                        res[oc][:], ALU.add, ALU.add)
                    sums.append(sm)
                return layernormT(sums, lng_col, lnb_col, eps, tag)

            L_EMIT = int(os.environ.get("K_LAYERS", L))
            SKIP_SA = bool(int(os.environ.get("K_SKIP_SA", "0")))
            SKIP_CA = bool(int(os.environ.get("K_SKIP_CA", "0")))
            for l in range(L_EMIT):
                # ===== self-attention =====
                wq = load_w(wd["sa_qw"], l, D, D)
                wk = load_w(wd["sa_kw"], l, D, D)
                wv = load_w(wd["sa_vw"], l, D, D)
                qT = linearT(wq, hT, D, TOK, _col("sa_qb", l, 0), "qT")
                kT = linearT(wk, hT, D, TOK, _col("sa_kb", l, 0), "kT")
                v_nat = []
                for t in range(SEQ_PER_CORE):
                    ps = pB.tile([128, D], F32, tag="m512")
                    for ic in range(NDC):
                        nc.tensor.matmul(ps[:], hT[ic][:, t * S:(t + 1) * S],
                                         wv[ic][:], start=(ic == 0),
                                         stop=(ic == NDC - 1))
                    vt = vpool.tile([128, D], F32R, tag="vnat")
                    nc.any.tensor_copy(vt[:], ps[:])
                    v_nat.append(vt)
                if SKIP_SA:
                    aT = qT
                else:
                    aT = attention(qT, kT, v_nat, k_sizes=[S],
                                   k_offs=[(0, S)], masked=True,
                                   vb_col=_col("sa_vb", l, 0))
                wo = load_w(wd["sa_ow"], l, D, D)
                hT = residual_ln(wo, aT, _col("sa_ob", l, 0), hT,
                                 _col("sa_lng", l, 0), _col("sa_lnb", l, 0),
                                 1e-8, "hT1")

                # ===== cross-attention =====
                wq = load_w(wd["ca_qw"], l, D, D)
                wk = load_w(wd["ca_kw"], l, D, D)
                wv = load_w(wd["ca_vw"], l, D, D)
                qT = linearT(wq, hT, D, TOK, _col("ca_qb", l, 0), "qT")
                kTm = linearT(wk, memT, D, SEQ_PER_CORE * M,
                              _col("ca_kb", l, 0), "kTm")
                ksz = [M // 2, M - M // 2]
                v_nat = []
                for b in range(SEQ_PER_CORE):
                    for j in range(2):
                        off = b * M + j * (M // 2)
                        sz = ksz[j]
                        ps = pB.tile([128, D], F32, tag="m512")
                        for ic in range(NDC):
                            nc.tensor.matmul(ps[0:sz, :],
                                             memT[ic][:, off:off + sz],
                                             wv[ic][:], start=(ic == 0),
                                             stop=(ic == NDC - 1))
                        vt = vpool.tile([128, D], F32R, tag="vnat")
                        nc.any.tensor_copy(vt[0:sz, :], ps[0:sz, :])
                        v_nat.append(vt)
                if SKIP_CA:
                    aT = qT
                else:
                    aT = attention(qT, kTm, v_nat, k_sizes=ksz,
                                   k_offs=[(0, M), (M // 2, M + M // 2)],
                                   masked=False, vb_col=_col("ca_vb", l, 0))
                wo = load_w(wd["ca_ow"], l, D, D)
                hT = residual_ln(wo, aT, _col("ca_ob", l, 0), hT,
                                 _col("ca_lng", l, 0), _col("ca_lnb", l, 0),
                                 1e-8, "hT2")

                # ===== feed-forward =====
                w1 = load_w(wd["ff_w1"], l, D, FF)
                ffT = []
                for oc in range(NFC):
                    ps = pA.tile([128, TOK], F32, tag="m256")
                    for ic in range(NDC):
                        nc.tensor.matmul(ps[:],
                                         w1[ic][:, oc * 128:(oc + 1) * 128],
                                         hT[ic][:], start=(ic == 0),
                                         stop=(ic == NDC - 1))
                    o = fpool.tile([128, TOK], F32R, tag="ffT")
                    cb = _col("ff_b1", l, oc)
                    nc.scalar.activation(o[:], ps[:], AF.Relu,
                                         bias=vecs[:, cb:cb + 1])
                    ffT.append(o)
                w2 = load_w(wd["ff_w2"], l, FF, D)
                hT = residual_ln(w2, ffT, _col("ff_b2", l, 0), hT,
                                 _col("ff_lng", l, 0), _col("ff_lnb", l, 0),
                                 1e-6, "hT3")

            dram = es.enter_context(tc.tile_pool(name="dram", bufs=1,
                                                 space="DRAM"))
            hcat = dram.tile([D, TOK], F32)
            for c in range(NDC):
                nc.sync.dma_start(hcat[c * 128:(c + 1) * 128, :],
                                  hT[c][:].bitcast(F32))

        # ---------------- all-gather + logits ----------------
        with tc.tile_pool(name="dram2", bufs=1, space="DRAM") as dram2:
            gath = dram2.tile([n_cores * D, TOK], F32)
            if n_cores > 1:
                nc.gpsimd.collective_compute(
                    "AllGather", ALU.bypass,
                    replica_groups=[list(range(n_cores))],
                    ins=[hcat[:].opt()], outs=[gath[:].opt()])
            else:
                nc.sync.dma_start(gath[0:D, :], hcat[:])

            with (
                tc.tile_pool(name="lwp", bufs=4) as lwp,
                tc.tile_pool(name="hallp", bufs=4) as hallp,
                tc.tile_pool(name="loutp", bufs=6) as loutp,
            ):
                n_tok_all = n_cores * TOK
                TW = 512 if n_tok_all % 512 == 0 else TOK
                hall = [hallp.tile([128, n_tok_all], F32R, tag="hall",
                         name=f"hall_{c}") for c in range(NDC)]
                for c in range(NDC):
                    for r in range(n_cores):
                        nc.sync.dma_start(
                            hall[c][:, r * TOK:(r + 1) * TOK],
                            gath[r * D + c * 128:r * D + (c + 1) * 128,
                                 :].bitcast(F32R))
                lw = []
                for ic in range(NDC):
                    t = lwp.tile([128, VSH], F32R, tag="lw")
                    nc.sync.dma_start(t[:], lw_d[ic * 128:(ic + 1) * 128, :])
                    lw.append(t)
                for vc in range(VSH // 128):
                    for t in range(n_tok_all // TW):
                        ps = pB.tile([128, TW], F32, tag="m512")
                        for ic in range(NDC):
                            nc.tensor.matmul(
                                ps[:], lw[ic][:, vc * 128:(vc + 1) * 128],
                                hall[ic][:, t * TW:(t + 1) * TW],
                                start=(ic == 0), stop=(ic == NDC - 1))
                        o = loutp.tile([128, TW], F32, tag="lo")
                        cb = _col("logit_b", 0, vc)
                        nc.any.tensor_scalar_add(o[:], ps[:],
                                                 vecs[:, cb:cb + 1])
                        nc.sync.dma_start(
                            out_d[vc * 128:(vc + 1) * 128,
                                  t * TW:(t + 1) * TW], o[:])


# ---------------------------------------------------------------------------
# host side
# ---------------------------------------------------------------------------
def _pack_vecs(inputs, core):
    v = np.zeros((128, NCOL), dtype=np.float32)
    for p in _PARAMS_D:
        arr = np.asarray(inputs[p], dtype=np.float32)        # [L, 512]
        for l in range(L):
            for c in range(NDC):
                v[:, _col(p, l, c)] = arr[l, c * 128:(c + 1) * 128]
    b1 = np.asarray(inputs["ff_b1"], dtype=np.float32)       # [L, 2048]
    for l in range(L):
        for c in range(NFC):
            v[:, _col("ff_b1", l, c)] = b1[l, c * 128:(c + 1) * 128]
    lb = np.asarray(inputs["logit_b"], dtype=np.float32)
    lbp = np.zeros(VPAD, dtype=np.float32)
    lbp[:V] = lb
    sh = lbp[core * VSH:(core + 1) * VSH]
    for c in range(VSH // 128):
        v[:, _col("logit_b", 0, c)] = sh[c * 128:(c + 1) * 128]
    return v


def prepare_in_maps(inputs, n_cores=N_CORES):
    x = np.asarray(inputs["x"])
    memory = np.asarray(inputs["memory"], dtype=np.float32)
    mask = np.asarray(inputs["mask"])
    embed = np.asarray(inputs["embed"], dtype=np.float32)
    pos = np.asarray(inputs["pos"], dtype=np.float32)

    h0 = embed[x] + pos[:S][None, :, :]                      # [B, S, D]
    lwp = np.zeros((D, VPAD), dtype=np.float32)
    lwp[:, :V] = np.asarray(inputs["logit_w"], dtype=np.float32)

    ones = np.ones((128, 1), dtype=np.float32)
    weights = {k: np.ascontiguousarray(np.asarray(inputs[k], dtype=np.float32))
               for k in ("sa_qw", "sa_kw", "sa_vw", "sa_ow",
                         "ca_qw", "ca_kw", "ca_vw", "ca_ow",
                         "ff_w1", "ff_w2")}

    in_maps = []
    for core in range(n_cores):
        b0 = core * SEQ_PER_CORE
        h0c = np.ascontiguousarray(h0[b0:b0 + SEQ_PER_CORE].reshape(TOK, D).T)
        memc = np.ascontiguousarray(
            memory[b0:b0 + SEQ_PER_CORE].reshape(SEQ_PER_CORE * M, D).T)
        mts = [np.ascontiguousarray(mask[b0 + b].T).astype(np.float32)
               for b in range(SEQ_PER_CORE)]
        mm = np.ascontiguousarray(np.concatenate([mts[0], mts[1]], axis=1))
        im = {
            "h0T": h0c, "memT": memc, "maskm": mm,
            "vecs": _pack_vecs(inputs, core), "ones": ones,
            "logit_w": np.ascontiguousarray(lwp[:, core * VSH:(core + 1) * VSH]),
        }
        im.update(weights)
        in_maps.append(im)
    return in_maps


@functools.cache
def _module():
    return build_module(N_CORES)


def kernel(**inputs):
    nc = _module()
    in_maps = prepare_in_maps(inputs, N_CORES)
    res = run_bass_kernel_spmd(nc, in_maps, core_ids=list(range(N_CORES)))
    outs = [res.results[c]["logitsT"] for c in range(N_CORES)]
    full = np.concatenate(outs, axis=0)[:V]                  # [32000, 2048]
    return np.ascontiguousarray(full.T).reshape(B, S, V)

